# revision 3
# baseline (speedup 1.0000x reference)
"""ConvAE Trainium2 kernel v3: conv(5x5)x3 + sparsity + deconv(13x13) + sigmoid.

Data-parallel over batch: 128 samples -> 8 NeuronCores x 16 samples.
Per-iteration tunnel traffic is the bottleneck on this axon setup (~430MB/s),
so v3 ships the replicated conv weights SHARDED 1/8-per-core and AllGathers
them on device, ships the compact 13x13 decoder kernel (43KB) instead of the
2MB expanded K16 (expanded on device), X in fp16, and returns uint8 sigmoid.
Compute: fp16 matmuls; software-pipelined decoder so the PE never idles.
Output written in block-deinterleaved uint8 layout; host reassembles to f32.
"""
import sys

sys.path.insert(0, "/opt/trn_rl_repo")

import numpy as np

import concourse.bass as bass
import concourse.bacc as bacc
import concourse.tile as tile
from concourse import bass_isa, mybir
from concourse.bass_utils import run_bass_kernel_spmd

AF = mybir.ActivationFunctionType
ALU = mybir.AluOpType
DT = mybir.dt

N_CORES = 8
B = 128
NSAMP = B // N_CORES  # 16
NF = 128

# packed fp16 weight layout (flat): W2T | W3T | WD  (W1T ships replicated
# so conv1 + im2col of early samples overlap the weight AllGather)
OFF_W2T = 0                      # [25,128,128] (tap, in, out)
OFF_W3T = OFF_W2T + 25 * 128 * 128
OFF_WD = OFF_W3T + 25 * 128 * 128  # [128,169] (ch, ky*13+kx)
WTOT = OFF_WD + 128 * 169        # 840832 = 8 * 105104
WSH = WTOT // N_CORES


def _chunks(total_rows, chunk_rows):
    out = []
    y = 0
    while y < total_rows:
        r = min(chunk_rows, total_rows - y)
        out.append((y, r))
        y += r
    return out


def _runs(s0):
    """Split ky in [0,13) into runs where (s0+ky)//4 is constant.

    Returns (ky0, n, p, r0): run start, length, block index, within-block start.
    """
    out = []
    k = 0
    while k < 13:
        a = s0 + k
        p, r0 = a // 4, a % 4
        n = min(4 - r0, 13 - k)
        out.append((k, n, p, r0))
        k += n
    return out


def build_program(nsamp=NSAMP):
    nc = bacc.Bacc()

    X16 = nc.declare_dram_parameter("X16", [nsamp, 4096], DT.float16, isOutput=False)
    W1T = nc.declare_dram_parameter("W1T", [25, 128], DT.float16, isOutput=False)
    WSHARD = nc.declare_dram_parameter("WSHARD", [1, WSH], DT.float16, isOutput=False)
    B1 = nc.declare_dram_parameter("B1", [128, 1], DT.float32, isOutput=False)
    B2 = nc.declare_dram_parameter("B2", [128, 1], DT.float32, isOutput=False)
    B3 = nc.declare_dram_parameter("B3", [128, 1], DT.float32, isOutput=False)
    BD16 = nc.declare_dram_parameter("BD16", [16, 1], DT.float32, isOutput=False)
    OUT = nc.declare_dram_parameter("OUT", [nsamp, 16 * 256], DT.uint8, isOutput=True)

    from contextlib import ExitStack
    with tile.TileContext(nc) as tc, ExitStack() as stk:
        dram = stk.enter_context(tc.tile_pool(name="dram", bufs=1, space="DRAM"))
        consts = stk.enter_context(tc.tile_pool(name="consts", bufs=1))
        p_im = stk.enter_context(tc.tile_pool(name="p_im", bufs=2))
        p_h1 = stk.enter_context(tc.tile_pool(name="p_h1", bufs=2))
        p_h2 = stk.enter_context(tc.tile_pool(name="p_h2", bufs=2))
        p_h3 = stk.enter_context(tc.tile_pool(name="p_h3", bufs=2))
        p_sp = stk.enter_context(tc.tile_pool(name="p_sp", bufs=2))
        p_out = stk.enter_context(tc.tile_pool(name="p_out", bufs=2))
        ps_conv = stk.enter_context(tc.tile_pool(name="ps_conv", bufs=3, space="PSUM"))
        ps_dec = stk.enter_context(tc.tile_pool(name="ps_dec", bufs=1, space="PSUM"))

        # ---- AllGather the weight shards: 1/8 per core -> full pack ----
        in_bounce = dram.tile([1, WSH], DT.float16, tag="in_bounce")
        out_bounce = dram.tile([N_CORES, WSH], DT.float16, tag="out_bounce")
        nc.sync.dma_start(out=in_bounce[:, :], in_=WSHARD[:, :])
        nc.gpsimd.collective_compute(
            "AllGather", ALU.bypass,
            replica_groups=[list(range(N_CORES))],
            ins=[in_bounce[:, :].opt()],
            outs=[out_bounce[:, :].opt()],
        )
        wt = out_bounce.tensor
        wbase = out_bounce[:, :].offset

        # ---- weights / consts into SBUF ----
        w1t = consts.tile([25, 128], DT.float16, tag="w1t")
        nc.sync.dma_start(out=w1t[:, :], in_=W1T[:, :])
        w2t = consts.tile([128, 25 * 128], DT.float16, tag="w2t")
        nc.sync.dma_start(
            out=w2t[:, :].rearrange("p (t o) -> p t o", t=25),
            in_=bass.AP(tensor=wt, offset=wbase + OFF_W2T,
                        ap=[[128, 128], [16384, 25], [1, 128]]))
        w3t = consts.tile([128, 25 * 128], DT.float16, tag="w3t")
        nc.sync.dma_start(
            out=w3t[:, :].rearrange("p (t o) -> p t o", t=25),
            in_=bass.AP(tensor=wt, offset=wbase + OFF_W3T,
                        ap=[[128, 128], [16384, 25], [1, 128]]))
        wdt = consts.tile([128, 169], DT.float16, tag="wdt")
        nc.sync.dma_start(
            out=wdt[:, :],
            in_=bass.AP(tensor=wt, offset=wbase + OFF_WD, ap=[[169, 128], [1, 169]]))
        b1t = consts.tile([128, 1], DT.float32, tag="b1t")
        nc.sync.dma_start(out=b1t[:, :], in_=B1[:, :])
        b2t = consts.tile([128, 1], DT.float32, tag="b2t")
        nc.sync.dma_start(out=b2t[:, :], in_=B2[:, :])
        b3t = consts.tile([128, 1], DT.float32, tag="b3t")
        nc.sync.dma_start(out=b3t[:, :], in_=B3[:, :])
        bd16 = consts.tile([16, 1], DT.float32, tag="bd16")
        nc.sync.dma_start(out=bd16[:, :], in_=BD16[:, :])

        # ---- expand wdt [128,169] -> k16t [128, 16*512] (slab layout) ----
        k16t = consts.tile([128, 16 * 512], DT.float16, tag="k16t")
        nc.vector.memset(k16t[:, :], 0.0)
        pdim = list(k16t[:, :].ap[0])
        spdim = list(wdt[:, :].ap[0])
        for sy in range(4):
            for sx in range(4):
                slab = sy * 4 + sx
                for (ky0, nky, py, ry0) in _runs(sy):
                    for (kx0, nkx, qx, tx0) in _runs(sx):
                        dst = bass.AP(
                            tensor=k16t.tensor,
                            offset=k16t[:, :].offset + slab * 512
                            + 32 * (py * 4 + qx) + ry0 * 4 + tx0,
                            ap=[pdim, [4, nky], [1, nkx]])
                        src = bass.AP(
                            tensor=wdt.tensor,
                            offset=wdt[:, :].offset + 13 * ky0 + kx0,
                            ap=[spdim, [13, nky], [1, nkx]])
                        nc.vector.tensor_scalar_mul(out=dst, in0=src, scalar1=1.0)

        hfins = [None] * nsamp

        def emit_convs_sparsity(s):
            # ---------------- conv1: [1,64,64] -> [128,60,60] ----------------
            im = p_im.tile([25, 3600], DT.float16, tag="im")
            for dy in range(5):
                nc.sync.dma_start(
                    out=im[dy * 5:(dy + 1) * 5, :].rearrange("b (y x) -> b y x", y=60),
                    in_=bass.AP(tensor=X16, offset=s * 4096 + dy * 64,
                                ap=[[1, 5], [64, 60], [1, 60]]),
                )
            h1 = p_h1.tile([128, 3600], DT.float16, tag="h1")
            for (y0, nr) in _chunks(60, 8):
                ps1 = ps_conv.tile([128, 480], DT.float32, tag="psc")
                n = nr * 60
                nc.tensor.matmul(ps1[:, 0:n], w1t[:, :],
                                 im[:, y0 * 60:(y0 + nr) * 60],
                                 start=True, stop=True)
                nc.scalar.activation(h1[:, y0 * 60:(y0 + nr) * 60], ps1[:, 0:n],
                                     AF.Relu, bias=b1t[:, 0:1], scale=1.0)

            # ---------------- conv2: -> [128,56,56] ----------------
            h2 = p_h2.tile([128, 3136], DT.float16, tag="h2")
            for (y0, nr) in _chunks(56, 8):
                ps2 = ps_conv.tile([128, 480], DT.float32, tag="psc")
                n = nr * 56
                for t in range(25):
                    dy, dx = t // 5, t % 5
                    rhs = bass.AP(tensor=h1.tensor,
                                  offset=h1[:, :].offset + (y0 + dy) * 60 + dx,
                                  ap=[list(h1[:, :].ap[0]), [60, nr], [1, 56]])
                    nc.tensor.matmul(ps2[:, 0:n], w2t[:, t * 128:(t + 1) * 128], rhs,
                                     start=(t == 0), stop=(t == 24))
                nc.scalar.activation(h2[:, y0 * 56:(y0 + nr) * 56], ps2[:, 0:n],
                                     AF.Relu, bias=b2t[:, 0:1], scale=1.0)

            # ---------------- conv3: -> [128,52,52] f32 (no relu) ------------
            h3 = p_h3.tile([128, 2704], DT.float32, tag="h3")
            for (y0, nr) in _chunks(52, 8):
                ps3 = ps_conv.tile([128, 480], DT.float32, tag="psc")
                n = nr * 52
                for t in range(25):
                    dy, dx = t // 5, t % 5
                    rhs = bass.AP(tensor=h2.tensor,
                                  offset=h2[:, :].offset + (y0 + dy) * 56 + dx,
                                  ap=[list(h2[:, :].ap[0]), [56, nr], [1, 52]])
                    nc.tensor.matmul(ps3[:, 0:n], w3t[:, t * 128:(t + 1) * 128], rhs,
                                     start=(t == 0), stop=(t == 24))
                nc.scalar.activation(h3[:, y0 * 52:(y0 + nr) * 52], ps3[:, 0:n],
                                     AF.Identity, bias=b3t[:, 0:1], scale=1.0)

            # ---------------- sparsity (DVE + gpsimd) ----------------
            m1 = p_sp.tile([128, 1], DT.float32, tag="m1")
            nc.vector.reduce_max(out=m1[:, :], in_=h3[:, :], axis=mybir.AxisListType.X)
            h3p = p_sp.tile([128, 2704], DT.float32, tag="h3p")
            nc.vector.scalar_tensor_tensor(out=h3p[:, :], in0=h3[:, :],
                                           scalar=m1[:, 0:1], in1=h3[:, :],
                                           op0=ALU.is_ge, op1=ALU.mult)
            # hierarchical 4x4-block max -> per-channel [128,169]
            t1 = p_sp.tile([128, 1352], DT.float32, tag="t1")  # [52,26]
            v = h3p[:, :].rearrange("p (y a two) -> p y a two", y=52, two=2)
            nc.vector.tensor_tensor(out=t1[:, :].rearrange("p (y a) -> p y a", y=52),
                                    in0=v[:, :, :, 0], in1=v[:, :, :, 1], op=ALU.max)
            t2 = p_sp.tile([128, 676], DT.float32, tag="t2")  # [52,13]
            v = t1[:, :].rearrange("p (y a two) -> p y a two", y=52, two=2)
            nc.vector.tensor_tensor(out=t2[:, :].rearrange("p (y a) -> p y a", y=52),
                                    in0=v[:, :, :, 0], in1=v[:, :, :, 1], op=ALU.max)
            t3 = p_sp.tile([128, 338], DT.float32, tag="t3")  # [26,13]
            v = t2[:, :].rearrange("p (a two x) -> p a two x", two=2, x=13)
            nc.vector.tensor_tensor(out=t3[:, :].rearrange("p (a x) -> p a x", x=13),
                                    in0=v[:, :, 0, :], in1=v[:, :, 1, :], op=ALU.max)
            t4 = p_sp.tile([128, 169], DT.float32, tag="t4")  # [13,13]
            v = t3[:, :].rearrange("p (a two x) -> p a two x", two=2, x=13)
            nc.vector.tensor_tensor(out=t4[:, :].rearrange("p (a x) -> p a x", x=13),
                                    in0=v[:, :, 0, :], in1=v[:, :, 1, :], op=ALU.max)
            Mb = p_sp.tile([128, 169], DT.float32, tag="Mb")
            nc.gpsimd.partition_all_reduce(Mb[:, :], t4[:, :], channels=128,
                                           reduce_op=bass_isa.ReduceOp.max)
            # keep = (h3p >= Mexp); hfin = keep * h3p (fp16 for decoder)
            kp = h3  # reuse conv3 tile (h3 fully consumed by m1/h3p)
            mb_ap = bass.AP(tensor=Mb.tensor, offset=Mb[:, :].offset,
                            ap=[list(Mb[:, :].ap[0]), [13, 13], [1, 13], [0, 4]])
            for sy in range(4):
                sl = bass.AP(tensor=h3p.tensor,
                             offset=h3p[:, :].offset + sy * 52,
                             ap=[list(h3p[:, :].ap[0]), [208, 13], [4, 13], [1, 4]])
                slo = bass.AP(tensor=kp.tensor,
                              offset=kp[:, :].offset + sy * 52,
                              ap=[list(kp[:, :].ap[0]), [208, 13], [4, 13], [1, 4]])
                nc.vector.tensor_tensor(out=slo, in0=sl, in1=mb_ap, op=ALU.is_ge)
            hfin = p_sp.tile([128, 2704], DT.float16, tag="hfin")
            nc.vector.tensor_tensor(out=hfin[:, :], in0=kp[:, :], in1=h3p[:, :],
                                    op=ALU.mult)
            hfins[s] = hfin

        def emit_decoder(s):
            hfin = hfins[s]
            psd = [ps_dec.tile([128, 169], DT.float32, tag=f"psd{i}",
                               name=f"psd{i}_{s}")
                   for i in range(4)]
            for slab in range(16):
                sy, sx = slab // 4, slab % 4
                rhs = bass.AP(tensor=hfin.tensor,
                              offset=hfin[:, :].offset + sy * 52 + sx,
                              ap=[list(hfin[:, :].ap[0]), [208, 13], [4, 13]])
                for ti in range(4):
                    nc.tensor.matmul(
                        psd[ti][:, :],
                        k16t[:, slab * 512 + ti * 128: slab * 512 + (ti + 1) * 128],
                        rhs, start=(slab == 0), stop=(slab == 15))
            out_d = p_out.tile([16, 256], DT.float32, tag="out_d")
            nc.vector.memset(out_d[:, :], 0.0)
            odv = out_d[:, :].rearrange("p (q w) -> p q w", q=16)
            for i in range(4):
                for j in range(4):
                    grp = i * 4 + j
                    ti, base = grp // 4, 32 * (grp % 4)
                    in1 = psd[ti][base:base + 16, :].rearrange("p (a b) -> p a b", a=13)
                    nc.vector.tensor_tensor(out=odv[:, i:i + 13, j:j + 13],
                                            in0=odv[:, i:i + 13, j:j + 13],
                                            in1=in1, op=ALU.add)
            sig = p_out.tile([16, 256], DT.float32, tag="sig")
            nc.scalar.activation(sig[:, :], out_d[:, :], AF.Sigmoid,
                                 bias=bd16[:, 0:1], scale=1.0)
            sig8 = p_out.tile([16, 256], DT.uint8, tag="sig8")
            nc.vector.tensor_scalar_mul(out=sig8[:, :], in0=sig[:, :], scalar1=255.0)
            nc.sync.dma_start(out=OUT[s, :], in_=sig8[:, :])

        for s in range(nsamp):
            emit_convs_sparsity(s)
            if s >= 1:
                emit_decoder(s - 1)
        emit_decoder(nsamp - 1)

    nc.finalize()
    return nc


def host_prep(w1, b1, w2, b2, w3, b3, wd, bd):
    w1 = np.asarray(w1, np.float32)
    w2 = np.asarray(w2, np.float32)
    w3 = np.asarray(w3, np.float32)
    wd = np.asarray(wd, np.float32)
    w1t = np.ascontiguousarray(w1.reshape(128, 25).T).astype(np.float16)
    pack = np.empty((WTOT,), np.float16)
    pack[OFF_W2T:OFF_W3T] = np.ascontiguousarray(
        w2.transpose(2, 3, 1, 0)).astype(np.float16).ravel()
    pack[OFF_W3T:OFF_WD] = np.ascontiguousarray(
        w3.transpose(2, 3, 1, 0)).astype(np.float16).ravel()
    pack[OFF_WD:] = wd[:, 0, :, :].reshape(128, 169).astype(np.float16).ravel()
    bdv = float(np.asarray(bd, np.float32).reshape(()))
    return {
        "pack": pack, "W1T": w1t,
        "B1": np.asarray(b1, np.float32).reshape(128, 1),
        "B2": np.asarray(b2, np.float32).reshape(128, 1),
        "B3": np.asarray(b3, np.float32).reshape(128, 1),
        "BD16": np.full((16, 1), bdv, np.float32),
    }


def make_in_maps(X, weights):
    X = np.asarray(X, np.float32)
    in_maps = []
    for c in range(N_CORES):
        xc = np.ascontiguousarray(
            X[c * NSAMP:(c + 1) * NSAMP].reshape(NSAMP, 4096)).astype(np.float16)
        in_maps.append({
            "X16": xc, "W1T": weights["W1T"],
            "WSHARD": weights["pack"][c * WSH:(c + 1) * WSH].reshape(1, WSH),
            "B1": weights["B1"], "B2": weights["B2"], "B3": weights["B3"],
            "BD16": weights["BD16"],
        })
    return in_maps


def assemble_out(results):
    out = np.empty((B, 1, 64, 64), np.float32)
    for c in range(N_CORES):
        ob = results[c]["OUT"].astype(np.float32) * (1.0 / 255.0)
        ob = ob.reshape(NSAMP, 4, 4, 16, 16)  # [n, r, s, q, p]
        # out[n, 4q+r, 4p+s] = ob[n, r, s, q, p]
        out[c * NSAMP:(c + 1) * NSAMP, 0] = (
            ob.transpose(0, 3, 1, 4, 2).reshape(NSAMP, 64, 64))
    return out


def run_cores(X, w1, b1, w2, b2, w3, b3, wd, bd, trace=False, **trace_kwargs):
    weights = host_prep(w1, b1, w2, b2, w3, b3, wd, bd)
    nc = build_program(NSAMP)
    in_maps = make_in_maps(X, weights)
    res = run_bass_kernel_spmd(nc, in_maps, core_ids=list(range(N_CORES)),
                               trace=trace, **trace_kwargs)
    return assemble_out(res.results), res


def kernel(X, w1, b1, w2, b2, w3, b3, wd, bd):
    out, _ = run_cores(X, w1, b1, w2, b2, w3, b3, wd, bd)
    return out
